# revision 18
# baseline (speedup 1.0000x reference)
"""ContinuousTimeRNN Trainium2 kernel.

Data-parallel over batch N=512 across 8 NeuronCores (64 rows each).
State h is kept transposed (H on partitions, batch on free dim).

Per core, per step t the recurrent matmul is computed W-stationary so the
result lands directly in the transposed layout (no PSUM->SBUF copy, no PE
transposes).  Each 128-row H-chunk m accumulates in its OWN PSUM bank so
its activation chain can start as soon as its group of matmuls finishes
(Tile serializes PE-writes vs engine-reads per bank):
    psum[m] = win3[m].T @ x3_t  +  sum_k wrec[k,m].T @ a_t[k]     (PE, bf16)
    h'[m]   = 0.9*h[m] + psum[m]                                  (DVE stt)
    th[m]   = tanh(h'[m])                                         (ACT, bf16 out)
    a'[m]   = max(th[m], 0)                                       (DVE bf16)
y_t = h' @ W_out (fp32r single-pass matmuls) is computed per 25-step
window from the h history buffer, interleaved into the FOLLOWING window's
steps so the matmuls fill PE stalls; the x input for the next body is
prefetched at the body tail (the loop back-edge barrier drains DMA).
"""

import sys

sys.path.insert(0, "/opt/trn_rl_repo")

import numpy as np
from ml_dtypes import bfloat16

ALPHA = 0.1
T, N, H, DIN, DOUT, INIT = 1000, 512, 512, 2, 2, 2
NCORES = 8
NS = N // NCORES          # 64 batch rows per core
WIN = 25                  # h-history window (steps)
NWIN = 8                  # windows per For_i body
BODY = NWIN * WIN         # steps per For_i body
NK = H // 128             # 4 H-chunks
YPAD = WIN * NS           # front pad columns of y (first bogus flush)
QW = WIN * NS // 5        # 320 y columns per quarter


def _build_nc(t_total=T):
    import concourse.mybir as mybir
    from concourse import bacc
    from concourse.tile import TileContext
    from concourse.bass import ds

    fp32 = mybir.dt.float32
    fp32r = mybir.dt.float32r
    bf16 = mybir.dt.bfloat16
    Alu = mybir.AluOpType
    Act = mybir.ActivationFunctionType

    nc = bacc.Bacc("TRN2", target_bir_lowering=False, debug=False,
                   num_devices=NCORES)

    n_bodies = t_total // BODY

    # -------- DRAM I/O (per core) --------
    wrec_d = nc.dram_tensor("wrec", [NK * NK, 128, 128], bf16,
                            kind="ExternalInput").ap()
    win3_d = nc.dram_tensor("win3", [DIN + 1, NK, 128], bf16,
                            kind="ExternalInput").ap()
    wout_d = nc.dram_tensor("wout", [NK, 128, DOUT], fp32r,
                            kind="ExternalInput").ap()
    fcw3_d = nc.dram_tensor("fcw3", [INIT + 1, NK, 128], fp32,
                            kind="ExternalInput").ap()
    init3_d = nc.dram_tensor("init3", [INIT + 1, NS], fp32,
                             kind="ExternalInput").ap()
    # padded by one body so the tail prefetch of the last body is in-range
    xt_d = nc.dram_tensor("xt", [DIN + 1, (t_total + BODY) * NS], bf16,
                          kind="ExternalInput").ap()
    y_d = nc.dram_tensor("y", [DOUT, YPAD + t_total * NS], fp32,
                         kind="ExternalOutput").ap()

    with TileContext(nc) as tc:
        with (
            tc.tile_pool(name="wpool", bufs=1) as wpool,
            tc.tile_pool(name="hpool", bufs=1) as hpool,
            tc.tile_pool(name="thpool", bufs=2) as thpool,
            tc.tile_pool(name="ypool", bufs=2) as ypool,
            tc.tile_pool(name="p3", bufs=1, space="PSUM") as p3pool,
            tc.tile_pool(name="py", bufs=2, space="PSUM") as pypool,
        ):
            # -------- persistent SBUF --------
            wrec_sb = wpool.tile([128, NK * NK, 128], bf16)  # 0.1*W_rec[k,m]
            win3_sb = wpool.tile([DIN + 1, NK, 128], bf16)   # 0.1*[W_in; bias]
            wout_sb = wpool.tile([128, NK, DOUT], fp32r)     # W_out chunks
            fcw3_sb = wpool.tile([INIT + 1, NK, 128], fp32)  # [fc_w.T; fc_b]
            init3_sb = wpool.tile([INIT + 1, NS], fp32)      # [initdir.T; ones]
            hist_a = hpool.tile([128, NK, WIN * NS], fp32r)
            hist_b = hpool.tile([128, NK, WIN * NS], fp32r)
            hist = [hist_a, hist_b]
            a_even = hpool.tile([128, NK * NS], bf16)        # a ping-pong
            a_odd = hpool.tile([128, NK * NS], bf16)
            abuf = [a_even, a_odd]
            xbuf = hpool.tile([DIN + 1, BODY * NS], bf16)    # per-body x

            nc.sync.dma_start(out=fcw3_sb[:], in_=fcw3_d)
            nc.sync.dma_start(out=init3_sb[:], in_=init3_d)
            nc.sync.dma_start(out=win3_sb[:], in_=win3_d)
            for j in range(NK * NK):
                nc.sync.dma_start(out=wrec_sb[:, j, :], in_=wrec_d[j])
            for k in range(NK):
                nc.sync.dma_start(out=wout_sb[:, k, :], in_=wout_d[k])
            nc.sync.dma_start(out=xbuf[:], in_=xt_d[:, 0:BODY * NS])

            # one PSUM bank per H-chunk (single-buffered)
            pch = [p3pool.tile([128, NS], fp32, tag=f"pch{m}",
                               name=f"pch{m}")
                   for m in range(NK)]

            # -------- h0 = fc(initdir) (transposed), a0 = relu(tanh(h0)) ----
            for m in range(NK):
                nc.tensor.matmul(pch[m][:], fcw3_sb[:, m, :], init3_sb[:],
                                 start=True, stop=True)
            for m in range(NK):
                nc.vector.tensor_copy(
                    hist[1][:, m, (WIN - 1) * NS: WIN * NS], pch[m][:])
            h0slot = hist[1][:, :, (WIN - 1) * NS: WIN * NS]
            th0 = thpool.tile([128, NK * NS], bf16, tag="th")
            nc.scalar.activation(th0[:], h0slot, Act.Tanh)
            nc.vector.tensor_relu(abuf[0][:], th0[:])

            ystate = {}

            def flush_mm(hsrc, q, k):
                """One y matmul (k-th contraction chunk of quarter q)."""
                if k == 0:
                    ystate["py"] = pypool.tile([DOUT, QW], fp32, tag="py",
                                               name="py")
                nc.tensor.matmul(ystate["py"][:], wout_sb[:, k, :],
                                 hsrc[:, k, q * QW:(q + 1) * QW],
                                 start=(k == 0), stop=(k == NK - 1))

            def flush_out(ycol):
                ysb = ypool.tile([DOUT, QW], fp32, tag="ysb", name="ysb")
                nc.scalar.copy(ysb[:], ystate["py"][:])
                nc.sync.dma_start(out=y_d[:, ycol], in_=ysb[:])

            def flush_quarter(hsrc, q, ycol):
                for k in range(NK):
                    flush_mm(hsrc, q, k)
                flush_out(ycol)

            # -------- time loop --------
            with tc.For_i(0, n_bodies * BODY, BODY,
                          hint_engines=(mybir.EngineType.PE,
                                        mybir.EngineType.DVE,
                                        mybir.EngineType.Activation)) as iv:
                for w in range(NWIN):
                    hc, hp = hist[w % 2], hist[1 - w % 2]
                    for s in range(WIN):
                        sg = w * WIN + s
                        a_cur = abuf[sg % 2]
                        a_nxt = abuf[1 - sg % 2]

                        xcol = sg * NS
                        # all four x MMs first: a-independent, and with one
                        # bank per chunk each start=True clears only its own
                        # bank, so they cover the a'-chain stall at the step
                        # boundary; rec k=0..3 then accumulate per bank.
                        for m in range(NK):
                            nc.tensor.matmul(pch[m][:], win3_sb[:, m, :],
                                             xbuf[:, xcol:xcol + NS],
                                             start=True, stop=False)
                        for m in range(NK):
                            for k in range(NK):
                                nc.tensor.matmul(pch[m][:],
                                                 wrec_sb[:, k * NK + m, :],
                                                 a_cur[:, k * NS:(k + 1) * NS],
                                                 start=False, stop=(k == NK - 1))

                        th = thpool.tile([128, NK * NS], bf16, tag="th")
                        for m in range(NK):
                            if s == 0:
                                prev_m = hp[:, m, (WIN - 1) * NS: WIN * NS]
                            else:
                                prev_m = hc[:, m, (s - 1) * NS: s * NS]
                            cur_m = hc[:, m, s * NS:(s + 1) * NS]
                            nc.vector.scalar_tensor_tensor(
                                out=cur_m, in0=prev_m, scalar=1.0 - ALPHA,
                                in1=pch[m][:],
                                op0=Alu.mult, op1=Alu.add)
                            nc.scalar.activation(
                                th[:, m * NS:(m + 1) * NS],
                                cur_m.bitcast(fp32), Act.Tanh)
                            nc.vector.tensor_relu(
                                a_nxt[:, m * NS:(m + 1) * NS],
                                th[:, m * NS:(m + 1) * NS])

                        # interleave the PREVIOUS window's y flush one matmul
                        # per step (fills the per-step PE stall).  Window w-1
                        # holds output rows iv+(w-1)*WIN..+24; for w==0 that
                        # is the previous body's last window (rows
                        # iv-WIN..iv-1), DMA offset YPAD+(iv-WIN)*NS ==
                        # iv*NS >= 0 (body 0 writes the discarded front pad).
                        if s < 20:
                            q, k = s // 4, s % 4
                            flush_mm(hp, q, k)
                            if k == NK - 1:
                                ycol = (ds(iv * NS + (w - 1) * WIN * NS + YPAD
                                           + q * QW, QW) if w > 0 else
                                        ds(iv * NS + q * QW, QW))
                                flush_out(ycol)

                # prefetch next body's x (back-edge barrier drains DMA, so
                # a body-top DMA would stall the PE ~3us)
                nc.sync.dma_start(out=xbuf[:],
                                  in_=xt_d[:, ds(iv * NS + BODY * NS,
                                                 BODY * NS)])

            # epilogue: flush the final window (rows T-WIN..T-1)
            for q in range(5):
                flush_quarter(hist[1], q,
                              ds(YPAD + (t_total - WIN) * NS + q * QW, QW))

    nc.compile()
    return nc


_NC_CACHE = {}


def _get_nc():
    if "nc" not in _NC_CACHE:
        _NC_CACHE["nc"] = _build_nc()
    return _NC_CACHE["nc"]


def _prep_in_maps(initdir, velocities, fc_w, fc_b, W_in, W_rec, W_out, bias):
    initdir = np.asarray(initdir, np.float32)
    velocities = np.asarray(velocities, np.float32)
    fc_w = np.asarray(fc_w, np.float32)
    fc_b = np.asarray(fc_b, np.float32)
    W_in = np.asarray(W_in, np.float32)
    W_rec = np.asarray(W_rec, np.float32)
    W_out = np.asarray(W_out, np.float32)
    bias = np.asarray(bias, np.float32)

    # host-side weight prep (shared across cores)
    w4 = (ALPHA * W_rec).reshape(NK, 128, NK, 128)
    wrec = np.ascontiguousarray(w4.transpose(0, 2, 1, 3)
                                ).reshape(NK * NK, 128, 128).astype(bfloat16)
    win3 = (ALPHA * np.concatenate([W_in, bias[None, :]], axis=0)
            ).reshape(DIN + 1, NK, 128).astype(bfloat16)
    wout = W_out.reshape(NK, 128, DOUT)
    fcw3 = np.concatenate([fc_w.T, fc_b[None, :]], axis=0
                          ).reshape(INIT + 1, NK, 128)

    in_maps = []
    for c in range(NCORES):
        sl = slice(c * NS, (c + 1) * NS)
        init3 = np.concatenate([initdir[sl].T,
                                np.ones((1, NS), np.float32)], axis=0)
        # xt[p, t*NS+n] = velocities[t, c*NS+n, p]; row DIN = ones;
        # one zero-padded body at the tail for the prefetch overrun
        xs = velocities[:, sl, :]                                # (T, NS, 2)
        xt = np.zeros((DIN + 1, (T + BODY) * NS), bfloat16)
        xt[:DIN, :T * NS] = (xs.transpose(2, 0, 1)
                             .reshape(DIN, T * NS).astype(bfloat16))
        xt[DIN, :T * NS] = bfloat16(1.0)
        in_maps.append({
            "wrec": np.ascontiguousarray(wrec),
            "win3": np.ascontiguousarray(win3),
            "wout": np.ascontiguousarray(wout),
            "fcw3": np.ascontiguousarray(fcw3),
            "init3": np.ascontiguousarray(init3),
            "xt": xt,
        })
    return in_maps


def _assemble(res):
    out = np.empty((T, N, DOUT), np.float32)
    for c in range(NCORES):
        yt = res.results[c]["y"][:, YPAD:]                       # (2, T*NS)
        out[:, c * NS:(c + 1) * NS, :] = (
            yt.reshape(DOUT, T, NS).transpose(1, 2, 0))
    return out


def kernel(initdir, velocities, fc_w, fc_b, W_in, W_rec, W_out, bias):
    from concourse.bass_utils import run_bass_kernel_spmd

    in_maps = _prep_in_maps(initdir, velocities, fc_w, fc_b, W_in, W_rec,
                            W_out, bias)
    nc = _get_nc()
    res = run_bass_kernel_spmd(nc, in_maps, list(range(NCORES)))
    return _assemble(res)


# revision 19
# speedup vs baseline: 1.1976x; 1.1976x over previous
"""ContinuousTimeRNN Trainium2 kernel.

Data-parallel over batch N=512 across 8 NeuronCores (64 rows each).
State h is kept transposed (H on partitions, batch on free dim).

Per core, per step t the recurrent matmul is computed W-stationary so the
result lands directly in the transposed layout (no PSUM->SBUF copy, no PE
transposes).  Each 128-row H-chunk m accumulates in its OWN PSUM bank so
its activation chain can start as soon as its group of matmuls finishes
(Tile serializes PE-writes vs engine-reads per bank):
    psum[m] = win3[m].T @ x3_t  +  sum_k wrec[k,m].T @ a_t[k]     (PE, bf16)
    h'[m]   = 0.9*h[m] + psum[m]                                  (DVE stt)
    th[m]   = tanh(h'[m])                                         (ACT, bf16 out)
    a'[m]   = max(th[m], 0)                                       (DVE bf16)
y_t = h' @ W_out (fp32r single-pass matmuls) is computed per 25-step
window from the h history buffer, interleaved into the FOLLOWING window's
steps so the matmuls fill PE stalls; the x input for the next body is
prefetched at the body tail (the loop back-edge barrier drains DMA).
"""

import sys

sys.path.insert(0, "/opt/trn_rl_repo")

import numpy as np
from ml_dtypes import bfloat16

ALPHA = 0.1
T, N, H, DIN, DOUT, INIT = 1000, 512, 512, 2, 2, 2
NCORES = 8
NS = N // NCORES          # 64 batch rows per core
WIN = 25                  # h-history window (steps)
NWIN = 8                  # windows per For_i body
BODY = NWIN * WIN         # steps per For_i body
NK = H // 128             # 4 H-chunks
YPAD = WIN * NS           # front pad columns of y (first bogus flush)
QW = WIN * NS // 4        # 400 y columns per quarter


def _build_nc(t_total=T):
    import concourse.mybir as mybir
    from concourse import bacc
    from concourse.tile import TileContext
    from concourse.bass import ds

    fp32 = mybir.dt.float32
    fp32r = mybir.dt.float32r
    bf16 = mybir.dt.bfloat16
    Alu = mybir.AluOpType
    Act = mybir.ActivationFunctionType

    nc = bacc.Bacc("TRN2", target_bir_lowering=False, debug=False,
                   num_devices=NCORES)

    n_bodies = t_total // BODY

    # -------- DRAM I/O (per core) --------
    wrec_d = nc.dram_tensor("wrec", [NK * NK, 128, 128], bf16,
                            kind="ExternalInput").ap()
    win3_d = nc.dram_tensor("win3", [DIN + 1, NK, 128], bf16,
                            kind="ExternalInput").ap()
    wout_d = nc.dram_tensor("wout", [NK, 128, DOUT], fp32r,
                            kind="ExternalInput").ap()
    fcw3_d = nc.dram_tensor("fcw3", [INIT + 1, NK, 128], fp32,
                            kind="ExternalInput").ap()
    init3_d = nc.dram_tensor("init3", [INIT + 1, NS], fp32,
                             kind="ExternalInput").ap()
    # padded by one body so the tail prefetch of the last body is in-range
    xt_d = nc.dram_tensor("xt", [DIN + 1, (t_total + BODY) * NS], bf16,
                          kind="ExternalInput").ap()
    y_d = nc.dram_tensor("y", [DOUT, YPAD + t_total * NS], fp32,
                         kind="ExternalOutput").ap()

    with TileContext(nc) as tc:
        with (
            tc.tile_pool(name="wpool", bufs=1) as wpool,
            tc.tile_pool(name="hpool", bufs=1) as hpool,
            tc.tile_pool(name="thpool", bufs=2) as thpool,
            tc.tile_pool(name="ypool", bufs=2) as ypool,
            tc.tile_pool(name="p3", bufs=1, space="PSUM") as p3pool,
            tc.tile_pool(name="py", bufs=2, space="PSUM") as pypool,
        ):
            # -------- persistent SBUF --------
            wrec_sb = wpool.tile([128, NK * NK, 128], bf16)  # 0.1*W_rec[k,m]
            win3_sb = wpool.tile([DIN + 1, NK, 128], bf16)   # 0.1*[W_in; bias]
            wout_sb = wpool.tile([128, NK, DOUT], fp32r)     # W_out chunks
            fcw3_sb = wpool.tile([INIT + 1, NK, 128], fp32)  # [fc_w.T; fc_b]
            init3_sb = wpool.tile([INIT + 1, NS], fp32)      # [initdir.T; ones]
            hist_a = hpool.tile([128, NK, WIN * NS], fp32r)
            hist_b = hpool.tile([128, NK, WIN * NS], fp32r)
            hist = [hist_a, hist_b]
            a_even = hpool.tile([128, NK * NS], bf16)        # a ping-pong
            a_odd = hpool.tile([128, NK * NS], bf16)
            abuf = [a_even, a_odd]
            xbuf = hpool.tile([DIN + 1, BODY * NS], bf16)    # per-body x

            nc.sync.dma_start(out=fcw3_sb[:], in_=fcw3_d)
            nc.sync.dma_start(out=init3_sb[:], in_=init3_d)
            nc.sync.dma_start(out=win3_sb[:], in_=win3_d)
            for j in range(NK * NK):
                nc.sync.dma_start(out=wrec_sb[:, j, :], in_=wrec_d[j])
            for k in range(NK):
                nc.sync.dma_start(out=wout_sb[:, k, :], in_=wout_d[k])
            nc.sync.dma_start(out=xbuf[:], in_=xt_d[:, 0:BODY * NS])

            # one PSUM bank per H-chunk (single-buffered)
            pch = [p3pool.tile([128, NS], fp32, tag=f"pch{m}",
                               name=f"pch{m}")
                   for m in range(NK)]

            # -------- h0 = fc(initdir) (transposed), a0 = relu(tanh(h0)) ----
            for m in range(NK):
                nc.tensor.matmul(pch[m][:], fcw3_sb[:, m, :], init3_sb[:],
                                 start=True, stop=True)
            for m in range(NK):
                nc.vector.tensor_copy(
                    hist[1][:, m, (WIN - 1) * NS: WIN * NS], pch[m][:])
            h0slot = hist[1][:, :, (WIN - 1) * NS: WIN * NS]
            th0 = thpool.tile([128, NK * NS], bf16, tag="th")
            nc.scalar.activation(th0[:], h0slot, Act.Tanh)
            nc.vector.tensor_relu(abuf[0][:], th0[:])

            ystate = {}

            def flush_mm(hsrc, q, k):
                """One y matmul (k-th contraction chunk of quarter q)."""
                if k == 0:
                    ystate["py"] = pypool.tile([DOUT, QW], fp32, tag="py",
                                               name="py")
                nc.tensor.matmul(ystate["py"][:], wout_sb[:, k, :],
                                 hsrc[:, k, q * QW:(q + 1) * QW],
                                 start=(k == 0), stop=(k == NK - 1))

            def flush_out(ycol):
                ysb = ypool.tile([DOUT, QW], fp32, tag="ysb", name="ysb")
                nc.scalar.copy(ysb[:], ystate["py"][:])
                nc.sync.dma_start(out=y_d[:, ycol], in_=ysb[:])

            def flush_quarter(hsrc, q, ycol):
                for k in range(NK):
                    flush_mm(hsrc, q, k)
                flush_out(ycol)

            # -------- time loop --------
            with tc.For_i(0, n_bodies * BODY, BODY,
                          hint_engines=(mybir.EngineType.PE,
                                        mybir.EngineType.DVE,
                                        mybir.EngineType.Activation)) as iv:
                for w in range(NWIN):
                    hc, hp = hist[w % 2], hist[1 - w % 2]
                    for s in range(WIN):
                        sg = w * WIN + s
                        a_cur = abuf[sg % 2]
                        a_nxt = abuf[1 - sg % 2]

                        xcol = sg * NS
                        # all four x MMs first: a-independent, and with one
                        # bank per chunk each start=True clears only its own
                        # bank, so they cover the a'-chain stall at the step
                        # boundary; rec k=0..3 then accumulate per bank.
                        for m in range(NK):
                            nc.tensor.matmul(pch[m][:], win3_sb[:, m, :],
                                             xbuf[:, xcol:xcol + NS],
                                             start=True, stop=False)
                        for m in range(NK):
                            for k in range(NK):
                                nc.tensor.matmul(pch[m][:],
                                                 wrec_sb[:, k * NK + m, :],
                                                 a_cur[:, k * NS:(k + 1) * NS],
                                                 start=False, stop=(k == NK - 1))

                        th = thpool.tile([128, NK * NS], bf16, tag="th")
                        for m in range(NK):
                            if s == 0:
                                prev_m = hp[:, m, (WIN - 1) * NS: WIN * NS]
                            else:
                                prev_m = hc[:, m, (s - 1) * NS: s * NS]
                            cur_m = hc[:, m, s * NS:(s + 1) * NS]
                            nc.vector.scalar_tensor_tensor(
                                out=cur_m, in0=prev_m, scalar=1.0 - ALPHA,
                                in1=pch[m][:],
                                op0=Alu.mult, op1=Alu.add)
                            nc.scalar.activation(
                                th[:, m * NS:(m + 1) * NS],
                                cur_m.bitcast(fp32), Act.Tanh)
                            nc.vector.tensor_relu(
                                a_nxt[:, m * NS:(m + 1) * NS],
                                th[:, m * NS:(m + 1) * NS])

                        # interleave the PREVIOUS window's y flush one matmul
                        # per step (fills the per-step PE stall).  Window w-1
                        # holds output rows iv+(w-1)*WIN..+24; for w==0 that
                        # is the previous body's last window (rows
                        # iv-WIN..iv-1), DMA offset YPAD+(iv-WIN)*NS ==
                        # iv*NS >= 0 (body 0 writes the discarded front pad).
                        if s < 16:
                            q, k = s // 4, s % 4
                            flush_mm(hp, q, k)
                            if k == NK - 1:
                                ycol = (ds(iv * NS + (w - 1) * WIN * NS + YPAD
                                           + q * QW, QW) if w > 0 else
                                        ds(iv * NS + q * QW, QW))
                                flush_out(ycol)

                # prefetch next body's x (back-edge barrier drains DMA, so
                # a body-top DMA would stall the PE ~3us)
                nc.sync.dma_start(out=xbuf[:],
                                  in_=xt_d[:, ds(iv * NS + BODY * NS,
                                                 BODY * NS)])

            # epilogue: flush the final window (rows T-WIN..T-1)
            for q in range(4):
                flush_quarter(hist[1], q,
                              ds(YPAD + (t_total - WIN) * NS + q * QW, QW))

    nc.compile()
    return nc


_NC_CACHE = {}


def _get_nc():
    if "nc" not in _NC_CACHE:
        _NC_CACHE["nc"] = _build_nc()
    return _NC_CACHE["nc"]


def _prep_in_maps(initdir, velocities, fc_w, fc_b, W_in, W_rec, W_out, bias):
    initdir = np.asarray(initdir, np.float32)
    velocities = np.asarray(velocities, np.float32)
    fc_w = np.asarray(fc_w, np.float32)
    fc_b = np.asarray(fc_b, np.float32)
    W_in = np.asarray(W_in, np.float32)
    W_rec = np.asarray(W_rec, np.float32)
    W_out = np.asarray(W_out, np.float32)
    bias = np.asarray(bias, np.float32)

    # host-side weight prep (shared across cores)
    w4 = (ALPHA * W_rec).reshape(NK, 128, NK, 128)
    wrec = np.ascontiguousarray(w4.transpose(0, 2, 1, 3)
                                ).reshape(NK * NK, 128, 128).astype(bfloat16)
    win3 = (ALPHA * np.concatenate([W_in, bias[None, :]], axis=0)
            ).reshape(DIN + 1, NK, 128).astype(bfloat16)
    wout = W_out.reshape(NK, 128, DOUT)
    fcw3 = np.concatenate([fc_w.T, fc_b[None, :]], axis=0
                          ).reshape(INIT + 1, NK, 128)

    in_maps = []
    for c in range(NCORES):
        sl = slice(c * NS, (c + 1) * NS)
        init3 = np.concatenate([initdir[sl].T,
                                np.ones((1, NS), np.float32)], axis=0)
        # xt[p, t*NS+n] = velocities[t, c*NS+n, p]; row DIN = ones;
        # one zero-padded body at the tail for the prefetch overrun
        xs = velocities[:, sl, :]                                # (T, NS, 2)
        xt = np.zeros((DIN + 1, (T + BODY) * NS), bfloat16)
        xt[:DIN, :T * NS] = (xs.transpose(2, 0, 1)
                             .reshape(DIN, T * NS).astype(bfloat16))
        xt[DIN, :T * NS] = bfloat16(1.0)
        in_maps.append({
            "wrec": np.ascontiguousarray(wrec),
            "win3": np.ascontiguousarray(win3),
            "wout": np.ascontiguousarray(wout),
            "fcw3": np.ascontiguousarray(fcw3),
            "init3": np.ascontiguousarray(init3),
            "xt": xt,
        })
    return in_maps


def _assemble(res):
    out = np.empty((T, N, DOUT), np.float32)
    for c in range(NCORES):
        yt = res.results[c]["y"][:, YPAD:]                       # (2, T*NS)
        out[:, c * NS:(c + 1) * NS, :] = (
            yt.reshape(DOUT, T, NS).transpose(1, 2, 0))
    return out


def kernel(initdir, velocities, fc_w, fc_b, W_in, W_rec, W_out, bias):
    from concourse.bass_utils import run_bass_kernel_spmd

    in_maps = _prep_in_maps(initdir, velocities, fc_w, fc_b, W_in, W_rec,
                            W_out, bias)
    nc = _get_nc()
    res = run_bass_kernel_spmd(nc, in_maps, list(range(NCORES)))
    return _assemble(res)


# revision 20
# speedup vs baseline: 1.1979x; 1.0003x over previous
"""ContinuousTimeRNN Trainium2 kernel.

Data-parallel over batch N=512 across 8 NeuronCores (64 rows each).
State h is kept transposed (H on partitions, batch on free dim).

Per core, per step t the recurrent matmul is computed W-stationary so the
result lands directly in the transposed layout (no PSUM->SBUF copy, no PE
transposes).  Each 128-row H-chunk m accumulates in its OWN PSUM bank so
its activation chain can start as soon as its group of matmuls finishes
(Tile serializes PE-writes vs engine-reads per bank):
    psum[m] = win3[m].T @ x3_t  +  sum_k wrec[k,m].T @ a_t[k]     (PE, bf16)
    h'[m]   = 0.9*h[m] + psum[m]                                  (DVE stt)
    th[m]   = tanh(h'[m])                                         (ACT, bf16 out)
    a'[m]   = max(th[m], 0)                                       (DVE bf16)
y_t = h' @ W_out (fp32r single-pass matmuls) is computed per 25-step
window from the h history buffer, interleaved into the FOLLOWING window's
steps so the matmuls fill PE stalls; the x input for the next body is
prefetched at the body tail (the loop back-edge barrier drains DMA).
"""

import sys

sys.path.insert(0, "/opt/trn_rl_repo")

import numpy as np
from ml_dtypes import bfloat16

ALPHA = 0.1
T, N, H, DIN, DOUT, INIT = 1000, 512, 512, 2, 2, 2
NCORES = 8
NS = N // NCORES          # 64 batch rows per core
WIN = 25                  # h-history window (steps)
NWIN = 8                  # windows per For_i body
BODY = NWIN * WIN         # steps per For_i body
NK = H // 128             # 4 H-chunks
YPAD = WIN * NS           # front pad columns of y (first bogus flush)
QW = WIN * NS // 4        # 400 y columns per quarter


def _build_nc(t_total=T):
    import concourse.mybir as mybir
    from concourse import bacc
    from concourse.tile import TileContext
    from concourse.bass import ds

    fp32 = mybir.dt.float32
    fp32r = mybir.dt.float32r
    bf16 = mybir.dt.bfloat16
    Alu = mybir.AluOpType
    Act = mybir.ActivationFunctionType

    nc = bacc.Bacc("TRN2", target_bir_lowering=False, debug=False,
                   num_devices=NCORES)

    n_bodies = t_total // BODY

    # -------- DRAM I/O (per core) --------
    wrec_d = nc.dram_tensor("wrec", [NK * NK, 128, 128], bf16,
                            kind="ExternalInput").ap()
    win3_d = nc.dram_tensor("win3", [DIN + 1, NK, 128], bf16,
                            kind="ExternalInput").ap()
    wout_d = nc.dram_tensor("wout", [NK, 128, DOUT], fp32r,
                            kind="ExternalInput").ap()
    fcw3_d = nc.dram_tensor("fcw3", [INIT + 1, NK, 128], fp32,
                            kind="ExternalInput").ap()
    init3_d = nc.dram_tensor("init3", [INIT + 1, NS], fp32,
                             kind="ExternalInput").ap()
    # padded by one body so the tail prefetch of the last body is in-range
    xt_d = nc.dram_tensor("xt", [DIN + 1, (t_total + BODY) * NS], bf16,
                          kind="ExternalInput").ap()
    y_d = nc.dram_tensor("y", [DOUT, YPAD + t_total * NS], fp32,
                         kind="ExternalOutput").ap()

    with TileContext(nc) as tc:
        with (
            tc.tile_pool(name="wpool", bufs=1) as wpool,
            tc.tile_pool(name="hpool", bufs=1) as hpool,
            tc.tile_pool(name="thpool", bufs=2) as thpool,
            tc.tile_pool(name="ypool", bufs=2) as ypool,
            tc.tile_pool(name="p3", bufs=1, space="PSUM") as p3pool,
            tc.tile_pool(name="py", bufs=2, space="PSUM") as pypool,
        ):
            # -------- persistent SBUF --------
            wrec_sb = wpool.tile([128, NK * NK, 128], bf16)  # 0.1*W_rec[k,m]
            win3_sb = wpool.tile([DIN + 1, NK, 128], bf16)   # 0.1*[W_in; bias]
            wout_sb = wpool.tile([128, NK, DOUT], fp32r)     # W_out chunks
            fcw3_sb = wpool.tile([INIT + 1, NK, 128], fp32)  # [fc_w.T; fc_b]
            init3_sb = wpool.tile([INIT + 1, NS], fp32)      # [initdir.T; ones]
            hist_a = hpool.tile([128, NK, WIN * NS], fp32r)
            hist_b = hpool.tile([128, NK, WIN * NS], fp32r)
            hist = [hist_a, hist_b]
            a_even = hpool.tile([128, NK * NS], bf16)        # a ping-pong
            a_odd = hpool.tile([128, NK * NS], bf16)
            abuf = [a_even, a_odd]
            xbuf = hpool.tile([DIN + 1, BODY * NS], bf16)    # per-body x

            nc.sync.dma_start(out=fcw3_sb[:], in_=fcw3_d)
            nc.sync.dma_start(out=init3_sb[:], in_=init3_d)
            nc.sync.dma_start(out=win3_sb[:], in_=win3_d)
            for j in range(NK * NK):
                nc.sync.dma_start(out=wrec_sb[:, j, :], in_=wrec_d[j])
            for k in range(NK):
                nc.sync.dma_start(out=wout_sb[:, k, :], in_=wout_d[k])
            nc.sync.dma_start(out=xbuf[:], in_=xt_d[:, 0:BODY * NS])

            # one PSUM bank per H-chunk; chunk 3 ping-pongs by step
            # parity so the next step's front-loaded x MM never waits on
            # the previous step's stt read of bank 3
            pch = [p3pool.tile([128, NS], fp32, tag=f"pch{m}",
                               name=f"pch{m}")
                   for m in range(NK)]
            pch3b = p3pool.tile([128, NS], fp32, tag="pch3b", name="pch3b")

            # -------- h0 = fc(initdir) (transposed), a0 = relu(tanh(h0)) ----
            for m in range(NK):
                nc.tensor.matmul(pch[m][:], fcw3_sb[:, m, :], init3_sb[:],
                                 start=True, stop=True)
            for m in range(NK):
                nc.vector.tensor_copy(
                    hist[1][:, m, (WIN - 1) * NS: WIN * NS], pch[m][:])
            h0slot = hist[1][:, :, (WIN - 1) * NS: WIN * NS]
            th0 = thpool.tile([128, NK * NS], bf16, tag="th")
            nc.scalar.activation(th0[:], h0slot, Act.Tanh)
            nc.vector.tensor_relu(abuf[0][:], th0[:])

            ystate = {}

            def flush_mm(hsrc, q, k):
                """One y matmul (k-th contraction chunk of quarter q)."""
                if k == 0:
                    ystate["py"] = pypool.tile([DOUT, QW], fp32, tag="py",
                                               name="py")
                nc.tensor.matmul(ystate["py"][:], wout_sb[:, k, :],
                                 hsrc[:, k, q * QW:(q + 1) * QW],
                                 start=(k == 0), stop=(k == NK - 1))

            def flush_out(ycol):
                ysb = ypool.tile([DOUT, QW], fp32, tag="ysb", name="ysb")
                nc.scalar.copy(ysb[:], ystate["py"][:])
                nc.sync.dma_start(out=y_d[:, ycol], in_=ysb[:])

            def flush_quarter(hsrc, q, ycol):
                for k in range(NK):
                    flush_mm(hsrc, q, k)
                flush_out(ycol)

            # -------- time loop --------
            with tc.For_i(0, n_bodies * BODY, BODY,
                          hint_engines=(mybir.EngineType.PE,
                                        mybir.EngineType.DVE,
                                        mybir.EngineType.Activation)) as iv:
                for w in range(NWIN):
                    hc, hp = hist[w % 2], hist[1 - w % 2]
                    for s in range(WIN):
                        sg = w * WIN + s
                        a_cur = abuf[sg % 2]
                        a_nxt = abuf[1 - sg % 2]
                        pc = [pch[0], pch[1], pch[2],
                              pch[3] if sg % 2 == 0 else pch3b]

                        xcol = sg * NS
                        # all four x MMs first: a-independent, and with one
                        # bank per chunk each start=True clears only its own
                        # bank, so they cover the a'-chain stall at the step
                        # boundary; rec k=0..3 then accumulate per bank.
                        for m in range(NK):
                            nc.tensor.matmul(pc[m][:], win3_sb[:, m, :],
                                             xbuf[:, xcol:xcol + NS],
                                             start=True, stop=False)
                        for m in range(NK):
                            for k in range(NK):
                                nc.tensor.matmul(pc[m][:],
                                                 wrec_sb[:, k * NK + m, :],
                                                 a_cur[:, k * NS:(k + 1) * NS],
                                                 start=False, stop=(k == NK - 1))

                        th = thpool.tile([128, NK * NS], bf16, tag="th")
                        for m in range(NK):
                            if s == 0:
                                prev_m = hp[:, m, (WIN - 1) * NS: WIN * NS]
                            else:
                                prev_m = hc[:, m, (s - 1) * NS: s * NS]
                            cur_m = hc[:, m, s * NS:(s + 1) * NS]
                            nc.vector.scalar_tensor_tensor(
                                out=cur_m, in0=prev_m, scalar=1.0 - ALPHA,
                                in1=pc[m][:],
                                op0=Alu.mult, op1=Alu.add)
                            nc.scalar.activation(
                                th[:, m * NS:(m + 1) * NS],
                                cur_m.bitcast(fp32), Act.Tanh)
                            nc.vector.tensor_relu(
                                a_nxt[:, m * NS:(m + 1) * NS],
                                th[:, m * NS:(m + 1) * NS])

                        # interleave the PREVIOUS window's y flush one matmul
                        # per step (fills the per-step PE stall).  Window w-1
                        # holds output rows iv+(w-1)*WIN..+24; for w==0 that
                        # is the previous body's last window (rows
                        # iv-WIN..iv-1), DMA offset YPAD+(iv-WIN)*NS ==
                        # iv*NS >= 0 (body 0 writes the discarded front pad).
                        if s < 16:
                            q, k = s // 4, s % 4
                            flush_mm(hp, q, k)
                            if k == NK - 1:
                                ycol = (ds(iv * NS + (w - 1) * WIN * NS + YPAD
                                           + q * QW, QW) if w > 0 else
                                        ds(iv * NS + q * QW, QW))
                                flush_out(ycol)

                # prefetch next body's x (back-edge barrier drains DMA, so
                # a body-top DMA would stall the PE ~3us)
                nc.sync.dma_start(out=xbuf[:],
                                  in_=xt_d[:, ds(iv * NS + BODY * NS,
                                                 BODY * NS)])

            # epilogue: flush the final window (rows T-WIN..T-1)
            for q in range(4):
                flush_quarter(hist[1], q,
                              ds(YPAD + (t_total - WIN) * NS + q * QW, QW))

    nc.compile()
    return nc


_NC_CACHE = {}


def _get_nc():
    if "nc" not in _NC_CACHE:
        _NC_CACHE["nc"] = _build_nc()
    return _NC_CACHE["nc"]


def _prep_in_maps(initdir, velocities, fc_w, fc_b, W_in, W_rec, W_out, bias):
    initdir = np.asarray(initdir, np.float32)
    velocities = np.asarray(velocities, np.float32)
    fc_w = np.asarray(fc_w, np.float32)
    fc_b = np.asarray(fc_b, np.float32)
    W_in = np.asarray(W_in, np.float32)
    W_rec = np.asarray(W_rec, np.float32)
    W_out = np.asarray(W_out, np.float32)
    bias = np.asarray(bias, np.float32)

    # host-side weight prep (shared across cores)
    w4 = (ALPHA * W_rec).reshape(NK, 128, NK, 128)
    wrec = np.ascontiguousarray(w4.transpose(0, 2, 1, 3)
                                ).reshape(NK * NK, 128, 128).astype(bfloat16)
    win3 = (ALPHA * np.concatenate([W_in, bias[None, :]], axis=0)
            ).reshape(DIN + 1, NK, 128).astype(bfloat16)
    wout = W_out.reshape(NK, 128, DOUT)
    fcw3 = np.concatenate([fc_w.T, fc_b[None, :]], axis=0
                          ).reshape(INIT + 1, NK, 128)

    in_maps = []
    for c in range(NCORES):
        sl = slice(c * NS, (c + 1) * NS)
        init3 = np.concatenate([initdir[sl].T,
                                np.ones((1, NS), np.float32)], axis=0)
        # xt[p, t*NS+n] = velocities[t, c*NS+n, p]; row DIN = ones;
        # one zero-padded body at the tail for the prefetch overrun
        xs = velocities[:, sl, :]                                # (T, NS, 2)
        xt = np.zeros((DIN + 1, (T + BODY) * NS), bfloat16)
        xt[:DIN, :T * NS] = (xs.transpose(2, 0, 1)
                             .reshape(DIN, T * NS).astype(bfloat16))
        xt[DIN, :T * NS] = bfloat16(1.0)
        in_maps.append({
            "wrec": np.ascontiguousarray(wrec),
            "win3": np.ascontiguousarray(win3),
            "wout": np.ascontiguousarray(wout),
            "fcw3": np.ascontiguousarray(fcw3),
            "init3": np.ascontiguousarray(init3),
            "xt": xt,
        })
    return in_maps


def _assemble(res):
    out = np.empty((T, N, DOUT), np.float32)
    for c in range(NCORES):
        yt = res.results[c]["y"][:, YPAD:]                       # (2, T*NS)
        out[:, c * NS:(c + 1) * NS, :] = (
            yt.reshape(DOUT, T, NS).transpose(1, 2, 0))
    return out


def kernel(initdir, velocities, fc_w, fc_b, W_in, W_rec, W_out, bias):
    from concourse.bass_utils import run_bass_kernel_spmd

    in_maps = _prep_in_maps(initdir, velocities, fc_w, fc_b, W_in, W_rec,
                            W_out, bias)
    nc = _get_nc()
    res = run_bass_kernel_spmd(nc, in_maps, list(range(NCORES)))
    return _assemble(res)
